# revision 1
# baseline (speedup 1.0000x reference)
"""TRN2 Bass kernel for nn_GAT_73950746902569 (GATv2 GNN, 8 NeuronCores).

Sharding: nodes/edges by target range (12544 padded nodes per core, 98 blocks
of 128 targets, KBLK edge-chunks of 128 per block).  Per layer: AllGather of
xl tables, segment-softmax attention via one-hot matmuls + indirect-DMA
gathers, BatchNorm via tiny AllReduce.  Pooling via one-hot matmul +
indirect scatter-add, ReduceScatter, then graph-sharded MLP.
"""
import numpy as np

import concourse.bass as bass
import concourse.bacc as bacc
import concourse.mybir as mybir
import concourse.tile as tile
from concourse.bass_utils import run_bass_kernel_spmd
from concourse.masks import make_identity

N, E, B = 100000, 200000, 4096
HID, EDIM, HEADS, L, NCLS = 256, 64, 8, 4, 3
M = 8
NPC = N // M
NB = 98
NPAD = NB * 128
GPC = B // M
BPAD = 4224
P = 128
F32 = mybir.dt.float32
I32 = mybir.dt.int32
AX = mybir.AxisListType.X
ALU = mybir.AluOpType
ACTF = mybir.ActivationFunctionType

_cache = {}


# ---------------------------------------------------------------- host prep
def _unpackbits_cols(a):
    bits = ((a[:, :, None] >> np.arange(7, -1, -1)) & 1)
    return bits.reshape(a.shape[0], -1).astype(np.float32)


def host_prep(inputs):
    x = np.asarray(inputs["x"])
    edge_index = np.asarray(inputs["edge_index"])
    edge_attr = np.asarray(inputs["edge_attr"])
    batch = np.asarray(inputs["batch"])

    src, tgt = edge_index[0], edge_index[1]
    order = np.argsort(tgt, kind="stable")
    srcs, tgts = src[order], tgt[order]
    a0s = edge_attr[order, 0].astype(np.float32)
    a1s = edge_attr[order, 1].astype(np.float32)
    src_pad = ((srcs // NPC) * NPAD + (srcs % NPC)).astype(np.int32)

    # per-core block boundaries: block b of core c covers targets
    # [c*NPC + 128*b, min(c*NPC + 128*(b+1), (c+1)*NPC))  (NPC % 128 != 0!)
    bounds = np.empty((M, NB + 1), np.int64)
    for c in range(M):
        for b in range(NB):
            bounds[c, b] = np.searchsorted(tgts, c * NPC + 128 * b)
        bounds[c, NB] = np.searchsorted(tgts, (c + 1) * NPC)
    blk_counts = (bounds[:, 1:] - bounds[:, :-1]).reshape(-1)
    KBLK = max(3, int(np.ceil(blk_counts.max() / 128)))
    NT = NB * KBLK

    cols_src = np.zeros((M, NT, 128, 1), np.int32)
    cols_trel = np.full((M, NT, 128, 1), 200.0, np.float32)
    rows_a = np.zeros((M, NT, 1, 256), np.float32)
    for c in range(M):
        for b in range(NB):
            s, e = bounds[c, b], bounds[c, b + 1]
            n = e - s
            for k in range((n + 127) // 128):
                ch = b * KBLK + k
                lo, hi = k * 128, min(n, (k + 1) * 128)
                mm = hi - lo
                cols_src[c, ch, :mm, 0] = src_pad[s + lo:s + hi]
                cols_trel[c, ch, :mm, 0] = tgts[s + lo:s + hi] - (c * NPC + b * 128)
                rows_a[c, ch, 0, :mm] = a0s[s + lo:s + hi]
                rows_a[c, ch, 0, 128:128 + mm] = a1s[s + lo:s + hi]

    bitsT = np.zeros((M, 56, NPAD), np.float32)
    x0col = np.zeros((M, NB, 128, 1), np.int32)
    for c in range(M):
        sl = slice(c * NPC, (c + 1) * NPC)
        bitsT[c, :, :NPC] = _unpackbits_cols(x[sl, 1:8]).T
        x0col[c, :, :, 0].reshape(-1)[:NPC] = x[sl, 0]

    brel = np.full((M, NB, 128, 1), 200.0, np.float32)
    pidx = np.zeros((M, NB, 128, 1), np.int32)
    for c in range(M):
        for b in range(NB):
            lo = c * NPC + b * 128
            hi = min(lo + 128, (c + 1) * NPC)
            gb = int(batch[lo])
            brel[c, b, :hi - lo, 0] = batch[lo:hi] - gb
            assert batch[hi - 1] - gb < 128
            pidx[c, b, :, 0] = gb + np.arange(128)

    mask97 = np.zeros((128, 1), np.float32)
    mask97[:NPC - (NB - 1) * 128] = 1.0

    def rep(v, n=128):
        v = np.asarray(v)
        return np.broadcast_to(v[None, :], (n, v.shape[-1])).astype(np.float32).copy()

    W = {}
    W["iota_full"] = np.broadcast_to(
        np.arange(128, dtype=np.float32)[None, :], (128, 128)).copy()
    W["atom_emb_pad"] = np.zeros((128, 128), np.float32)
    W["atom_emb_pad"][:120] = np.asarray(inputs["atom_emb"])
    W["atom_lin_w"] = np.asarray(inputs["atom_lin_w"], np.float32)
    W["atom_lin_b_rep"] = rep(inputs["atom_lin_b"])
    W["edge_embT"] = np.zeros((64, 32), np.float32)
    W["edge_embT"][:, :22] = np.asarray(inputs["edge_emb"]).T
    W["edge_emb_pad"] = np.zeros((32, 64), np.float32)
    W["edge_emb_pad"][:22] = np.asarray(inputs["edge_emb"])
    W["b22T"] = np.zeros((8, 32), np.float32)
    W["b22T"][:, :22] = _unpackbits_cols(np.arange(22)[:, None]).T
    W["edge_lin_w"] = np.asarray(inputs["edge_lin_w"], np.float32)
    W["edge_lin_b_rep"] = rep(inputs["edge_lin_b"], 32)
    W["lin_l_w"] = np.asarray(inputs["lin_l_w"], np.float32)
    W["lin_r_w"] = np.asarray(inputs["lin_r_w"], np.float32)
    W["lin_e_w"] = np.asarray(inputs["lin_e_w"], np.float32)
    W["lin_l_b_rep"] = np.stack([rep(np.asarray(inputs["lin_l_b"])[l]) for l in range(L)])
    W["lin_r_b_rep"] = np.stack([rep(np.asarray(inputs["lin_r_b"])[l]) for l in range(L)])
    W["att_rep"] = np.stack([rep(np.asarray(inputs["att"])[l]) for l in range(L)])
    W["conv_b_rep"] = np.stack([rep(np.asarray(inputs["conv_b"])[l]) for l in range(L)])
    W["bn_g"] = np.asarray(inputs["bn_g"], np.float32)[:, None, :]
    W["bn_b"] = np.asarray(inputs["bn_b"], np.float32)[:, None, :]
    for k in ("w1", "w2", "w3", "w4"):
        W[k] = np.asarray(inputs[k], np.float32)
    for k in ("b1", "b2", "b3", "b4"):
        W[k + "_rep"] = rep(inputs[k])
    W["mask97"] = mask97

    in_maps = []
    for c in range(M):
        im = dict(W)
        im["cols_src"] = cols_src[c]
        im["cols_trel"] = cols_trel[c]
        im["rows_a"] = rows_a[c]
        im["bitsT"] = bitsT[c]
        im["x0col"] = x0col[c]
        im["brel"] = brel[c]
        im["pidx"] = pidx[c]
        in_maps.append(im)
    return in_maps, KBLK


# ---------------------------------------------------------------- device build
def build(KBLK, nlayers=L, debug=False):
    NT = NB * KBLK
    nc = bacc.Bacc("TRN2", target_bir_lowering=False, debug=False,
                   enable_asserts=False, num_devices=M)

    def din(name, shape, dt=F32):
        return nc.dram_tensor(name, list(shape), dt, kind="ExternalInput").ap()

    t_cols_src = din("cols_src", [NT, 128, 1], I32)
    t_cols_trel = din("cols_trel", [NT, 128, 1])
    t_rows_a = din("rows_a", [NT, 1, 256])
    t_bitsT = din("bitsT", [56, NPAD])
    t_x0col = din("x0col", [NB, 128, 1], I32)
    t_brel = din("brel", [NB, 128, 1])
    t_pidx = din("pidx", [NB, 128, 1], I32)
    t_mask97 = din("mask97", [128, 1])
    t_iota = din("iota_full", [128, 128])
    t_aemb = din("atom_emb_pad", [128, 128])
    t_alw = din("atom_lin_w", [56, 128])
    t_alb = din("atom_lin_b_rep", [128, 128])
    t_eembT = din("edge_embT", [64, 32])
    t_eemb = din("edge_emb_pad", [32, 64])
    t_b22T = din("b22T", [8, 32])
    t_elw = din("edge_lin_w", [8, 64])
    t_elb = din("edge_lin_b_rep", [32, 64])
    t_llw = din("lin_l_w", [L, 256, 256])
    t_lrw = din("lin_r_w", [L, 256, 256])
    t_lew = din("lin_e_w", [L, 128, 256])
    t_llb = din("lin_l_b_rep", [L, 128, 256])
    t_lrb = din("lin_r_b_rep", [L, 128, 256])
    t_att = din("att_rep", [L, 128, 256])
    t_cvb = din("conv_b_rep", [L, 128, 256])
    t_bng = din("bn_g", [L, 1, 256])
    t_bnb = din("bn_b", [L, 1, 256])
    t_w1 = din("w1", [256, 1024])
    t_w2 = din("w2", [1024, 1024])
    t_w3 = din("w3", [1024, 512])
    t_w4 = din("w4", [512, NCLS])
    t_b1 = din("b1_rep", [128, 1024])
    t_b2 = din("b2_rep", [128, 1024])
    t_b3 = din("b3_rep", [128, 512])
    t_b4 = din("b4_rep", [128, NCLS])

    out_y = nc.dram_tensor("out_y", [GPC, NCLS], F32, kind="ExternalOutput").ap()
    dbg = {}

    def dout(name, shape):
        ap = nc.dram_tensor(name, list(shape), F32, kind="ExternalOutput").ap()
        dbg[name] = ap
        return ap

    if debug:
        dout("dbg_v", [3, 128, 256])
        dout("dbg_y", [3, 128, 256])
        dout("dbg_p", [3, 128, HEADS])
        dout("dbg_alpha", [3, 128, HEADS])
        dout("dbg_nd", [128, 256 + HEADS])
        dout("dbg_xls", [3, 128, 256])
        dout("dbg_xl0", [NPAD, 256])
        dout("dbg_xr0", [NPAD, 256])
        dout("dbg_lft", [NPAD, 128])
        dout("dbg_out", [NPAD, 256])
        dout("dbg_pool", [BPAD, 257])

    with tile.TileContext(nc) as tc:
        with (
            tc.tile_pool(name="const", bufs=1) as cst,
            tc.tile_pool(name="dram", bufs=1, space="DRAM") as dram,
        ):
            d_xl = dram.tile([NPAD, 256], F32)
            d_xr = dram.tile([NPAD, 256], F32)
            d_xl_alls = [dram.tile([M * NPAD, 256], F32, addr_space="Shared",
                                   name=f"d_xl_all{i}") for i in range(L)]
            d_out = dram.tile([NPAD, 256], F32)
            d_lft = dram.tile([NPAD, 128], F32)
            d_pool = dram.tile([BPAD, 257], F32)
            d_pool_rs = dram.tile([GPC, 257], F32)
            d_st_in = dram.tile([1, 512], F32)
            d_st_outs = [dram.tile([1, 512], F32, addr_space="Shared",
                                   name=f"d_st_out{i}") for i in range(L)]
            d_aemb = dram.tile([128, 128], F32)

            # ---------------- resident constants ----------------
            iota_f = cst.tile([128, 128], F32)
            nc.sync.dma_start(iota_f[:], t_iota[:])
            ident = cst.tile([128, 128], F32)
            make_identity(nc, ident[:])
            mask97 = cst.tile([128, 1], F32)
            nc.sync.dma_start(mask97[:], t_mask97[:])
            ones1 = cst.tile([1, 128], F32)
            nc.any.memset(ones1[:], 1.0)
            onesc = cst.tile([128, 1], F32)
            nc.any.memset(onesc[:], 1.0)
            iotac = cst.tile([128, 1], F32)
            iota32c = cst.tile([32, 1], F32)
            att_sb = cst.tile([128, L * 256], F32)
            cvb_sb = cst.tile([128, L * 256], F32)
            llb_sb = cst.tile([128, L * 256], F32)
            lrb_sb = cst.tile([128, L * 256], F32)
            lew_sb = cst.tile([128, L * 256], F32)
            llw_sb = cst.tile([128, 2 * L * 256], F32)
            lrw_sb = cst.tile([128, 2 * L * 256], F32)
            for l in range(L):
                sl = slice(l * 256, (l + 1) * 256)
                nc.sync.dma_start(att_sb[:, sl], t_att[l])
                nc.sync.dma_start(cvb_sb[:, sl], t_cvb[l])
                nc.sync.dma_start(llb_sb[:, sl], t_llb[l])
                nc.sync.dma_start(lrb_sb[:, sl], t_lrb[l])
                nc.sync.dma_start(lew_sb[:, sl], t_lew[l])
                for h in range(2):
                    s2 = slice((2 * l + h) * 256, (2 * l + h + 1) * 256)
                    nc.sync.dma_start(llw_sb[:, s2], t_llw[l, 128 * h:128 * (h + 1), :])
                    nc.sync.dma_start(lrw_sb[:, s2], t_lrw[l, 128 * h:128 * (h + 1), :])
            t2_sb = cst.tile([32, 64], F32)
            eemb_sb = cst.tile([32, 64], F32)
            nc.sync.dma_start(eemb_sb[:], t_eemb[:])
            lew_lo = cst.tile([64, L * 256], F32)
            for l in range(L):
                nc.sync.dma_start(lew_lo[:, l * 256:(l + 1) * 256], t_lew[l, 64:128, :])
            EA = cst.tile([32, L * 256], F32)
            EB = cst.tile([32, L * 256], F32)
            nc.sync.dma_start(d_aemb[:], t_aemb[:])

            def headb(ap_t, Hn, Cn):
                a = ap_t[:]
                return bass.AP(a.tensor, a.offset, [a.ap[0], [a.ap[1][0], Hn], [0, Cn]])

            def emit_xlxr(xh, l, b, ps, sb):
                ws = []
                for h in range(2):
                    tp = ps.tile([128, 128], F32, space="PSUM", tag="tpos")
                    nc.tensor.transpose(tp[:], xh[:, 128 * h:128 * (h + 1)], ident[:])
                    tsb = sb.tile([128, 128], F32, tag="tsb")
                    nc.scalar.copy(tsb[:], tp[:])
                    ws.append(tsb)
                xl_ps = ps.tile([128, 256], F32, space="PSUM", tag="xlps")
                xr_ps = ps.tile([128, 256], F32, space="PSUM", tag="xrps")
                for h in range(2):
                    s2 = slice((2 * l + h) * 256, (2 * l + h + 1) * 256)
                    nc.tensor.matmul(xl_ps[:], lhsT=ws[h][:], rhs=llw_sb[:, s2],
                                     start=(h == 0), stop=(h == 1))
                    nc.tensor.matmul(xr_ps[:], lhsT=ws[h][:], rhs=lrw_sb[:, s2],
                                     start=(h == 0), stop=(h == 1))
                xl_sb = sb.tile([128, 256], F32, tag="xlsb")
                xr_sb = sb.tile([128, 256], F32, tag="xrsb")
                lsl = slice(l * 256, (l + 1) * 256)
                nc.vector.tensor_add(xl_sb[:], xl_ps[:], llb_sb[:, lsl])
                nc.vector.tensor_add(xr_sb[:], xr_ps[:], lrb_sb[:, lsl])
                rsl = slice(b * 128, (b + 1) * 128)
                nc.sync.dma_start(d_xl[rsl, :], xl_sb[:])
                nc.sync.dma_start(d_xr[rsl, :], xr_sb[:])

            # ---------------- featurization ----------------
            with (
                tc.tile_pool(name="psF", bufs=2, space="PSUM") as ps,
                tc.tile_pool(name="sbF", bufs=3) as sb,
            ):
                io_ps = ps.tile([128, 128], F32, space="PSUM", tag="tpos")
                nc.tensor.transpose(io_ps[:], iota_f[:], ident[:])
                nc.vector.tensor_copy(iotac[:], io_ps[:, :1])
                nc.vector.tensor_copy(iota32c[:], io_ps[:32, :1])

                elw_sb = sb.tile([8, 64], F32)
                nc.sync.dma_start(elw_sb[:], t_elw[:])
                b22T_sb = sb.tile([8, 32], F32)
                nc.sync.dma_start(b22T_sb[:], t_b22T[:])
                elb_sb = sb.tile([32, 64], F32)
                nc.sync.dma_start(elb_sb[:], t_elb[:])
                t2_ps = ps.tile([32, 64], F32, space="PSUM", tag="xlps")
                nc.tensor.matmul(t2_ps[:], lhsT=b22T_sb[:], rhs=elw_sb[:],
                                 start=True, stop=True)
                nc.vector.tensor_add(t2_sb[:], t2_ps[:], elb_sb[:])
                t2t_ps = ps.tile([64, 32], F32, space="PSUM", tag="xrps")
                nc.tensor.transpose(t2t_ps[:], t2_sb[:], ident[:32, :32])
                t2t_sb = sb.tile([64, 32], F32)
                nc.vector.tensor_copy(t2t_sb[:], t2t_ps[:])
                eembT_sb = sb.tile([64, 32], F32)
                nc.sync.dma_start(eembT_sb[:], t_eembT[:])
                for l in range(nlayers):
                    sl = slice(l * 256, (l + 1) * 256)
                    ea_ps = ps.tile([32, 256], F32, space="PSUM", tag="xlps")
                    nc.tensor.matmul(ea_ps[:], lhsT=eembT_sb[:], rhs=lew_sb[:64, sl],
                                     start=True, stop=True)
                    nc.vector.tensor_copy(EA[:, sl], ea_ps[:])
                    eb_ps = ps.tile([32, 256], F32, space="PSUM", tag="xrps")
                    nc.tensor.matmul(eb_ps[:], lhsT=t2t_sb[:], rhs=lew_lo[:, sl],
                                     start=True, stop=True)
                    nc.vector.tensor_copy(EB[:, sl], eb_ps[:])

                alw_sb = sb.tile([56, 128], F32)
                nc.sync.dma_start(alw_sb[:], t_alw[:])
                alb_sb = sb.tile([128, 128], F32)
                nc.sync.dma_start(alb_sb[:], t_alb[:])
                for b in range(NB):
                    x0_t = sb.tile([128, 1], I32, tag="x0")
                    nc.sync.dma_start(x0_t[:], t_x0col[b])
                    xf = sb.tile([128, 256], F32, tag="xf")
                    nc.gpsimd.indirect_dma_start(
                        out=xf[:, :128], out_offset=None, in_=d_aemb.opt(),
                        in_offset=bass.IndirectOffsetOnAxis(ap=x0_t[:, :1], axis=0))
                    bt = sb.tile([56, 128], F32, tag="bits")
                    nc.sync.dma_start(bt[:], t_bitsT[:, b * 128:(b + 1) * 128])
                    x2_ps = ps.tile([128, 128], F32, space="PSUM", tag="tpos")
                    nc.tensor.matmul(x2_ps[:], lhsT=bt[:], rhs=alw_sb[:],
                                     start=True, stop=True)
                    nc.vector.tensor_add(xf[:, 128:], x2_ps[:], alb_sb[:])
                    emit_xlxr(xf, 0, b, ps, sb)

            # loop_fT (separate PSUM scope to fit the 8-bank budget)
            with (
                tc.tile_pool(name="psF2", bufs=2, space="PSUM") as ps,
                tc.tile_pool(name="psF1", bufs=1, space="PSUM") as ps1f,
                tc.tile_pool(name="sbF2", bufs=3) as sb,
            ):
                for b in range(NB):
                    lsum_ps = ps.tile([128, 128], F32, space="PSUM", tag="lsum")
                    cnt_ps = ps.tile([128, 1], F32, space="PSUM", tag="cnt")
                    for k in range(KBLK):
                        ch = b * KBLK + k
                        trel = sb.tile([128, 1], F32, tag="trel")
                        nc.sync.dma_start(trel[:], t_cols_trel[ch])
                        rowa = sb.tile([1, 256], F32, tag="rowa")
                        nc.sync.dma_start(rowa[:], t_rows_a[ch])
                        rep_ps = ps1f.tile([32, 256], F32, space="PSUM", tag="reps")
                        nc.tensor.matmul(rep_ps[:], lhsT=ones1[:, :32], rhs=rowa[:],
                                         start=True, stop=True)
                        oh0 = sb.tile([32, 128], F32, tag="oh0")
                        nc.vector.tensor_scalar(out=oh0[:], in0=rep_ps[:, :128],
                                                scalar1=iota32c[:, :1], scalar2=None,
                                                op0=ALU.is_equal)
                        oh1 = sb.tile([32, 128], F32, tag="oh1")
                        nc.vector.tensor_scalar(out=oh1[:], in0=rep_ps[:, 128:],
                                                scalar1=iota32c[:, :1], scalar2=None,
                                                op0=ALU.is_equal)
                        st = sb.tile([128, 128], F32, tag="st")
                        nc.vector.tensor_scalar(out=st[:], in0=iota_f[:],
                                                scalar1=trel[:, :1], scalar2=None,
                                                op0=ALU.is_equal)
                        ef_ps = ps1f.tile([128, 128], F32, space="PSUM", tag="efps")
                        nc.tensor.matmul(ef_ps[:, :64], lhsT=oh0[:], rhs=eemb_sb[:],
                                         start=True, stop=True)
                        nc.tensor.matmul(ef_ps[:, 64:], lhsT=oh1[:], rhs=t2_sb[:],
                                         start=True, stop=True)
                        ef_sb = sb.tile([128, 128], F32, tag="efsb")
                        nc.scalar.copy(ef_sb[:], ef_ps[:])
                        nc.tensor.matmul(lsum_ps[:], lhsT=st[:], rhs=ef_sb[:],
                                         start=(k == 0), stop=(k == KBLK - 1))
                        nc.tensor.matmul(cnt_ps[:], lhsT=st[:], rhs=onesc[:],
                                         start=(k == 0), stop=(k == KBLK - 1))
                    cnt_m = sb.tile([128, 1], F32, tag="cntm")
                    nc.vector.tensor_scalar(out=cnt_m[:], in0=cnt_ps[:], scalar1=1.0,
                                            scalar2=None, op0=ALU.max)
                    rcnt = sb.tile([128, 1], F32, tag="rcnt")
                    nc.vector.reciprocal(rcnt[:], cnt_m[:])
                    lf = sb.tile([128, 128], F32, tag="lf")
                    nc.vector.tensor_scalar(out=lf[:], in0=lsum_ps[:],
                                            scalar1=rcnt[:, :1], scalar2=None,
                                            op0=ALU.mult)
                    lft_ps = ps1f.tile([128, 128], F32, space="PSUM", tag="lftps")
                    nc.tensor.transpose(lft_ps[:], lf[:], ident[:])
                    lft_sb = sb.tile([128, 128], F32, tag="lftsb")
                    nc.scalar.copy(lft_sb[:], lft_ps[:])
                    nc.sync.dma_start(d_lft[b * 128:(b + 1) * 128, :], lft_sb[:])

            if debug:
                nc.sync.dma_start(dbg["dbg_xl0"][:], d_xl.opt())
                nc.sync.dma_start(dbg["dbg_xr0"][:], d_xr.opt())
                nc.sync.dma_start(dbg["dbg_lft"][:], d_lft.opt())

            # ---------------- conv layers ----------------
            for l in range(nlayers):
                H = HEADS if l == 0 else 1
                C = HID // H
                lsl = slice(l * 256, (l + 1) * 256)
                d_xl_all = d_xl_alls[l]
                nc.gpsimd.collective_compute(
                    "AllGather", ALU.bypass, ins=[d_xl.opt()], outs=[d_xl_all.opt()],
                    replica_groups=[list(range(M))])

                with (
                    tc.tile_pool(name=f"ps2L{l}", bufs=2, space="PSUM") as ps2,
                    tc.tile_pool(name=f"ps1L{l}", bufs=1, space="PSUM") as ps1,
                    tc.tile_pool(name=f"psStL{l}", bufs=1, space="PSUM") as psst,
                    tc.tile_pool(name=f"sbL{l}", bufs=3) as sb,
                ):
                    sum_ps = psst.tile([1, 256], F32, space="PSUM", tag="sum")
                    sq_ps = psst.tile([1, 256], F32, space="PSUM", tag="sq")
                    for b in range(NB):
                        nd_ps = ps2.tile([128, 256], F32, space="PSUM", tag="nd")
                        den_ps = ps1.tile([128, H], F32, space="PSUM", tag="den")
                        xr_blk = sb.tile([128, 256], F32, tag="xrblk")
                        nc.sync.dma_start(xr_blk[:], d_xr[b * 128:(b + 1) * 128, :])
                        for k in range(KBLK):
                            ch = b * KBLK + k
                            srcc = sb.tile([128, 1], I32, tag="srcc")
                            nc.sync.dma_start(srcc[:], t_cols_src[ch])
                            trel = sb.tile([128, 1], F32, tag="trel")
                            nc.sync.dma_start(trel[:], t_cols_trel[ch])
                            rowa = sb.tile([1, 256], F32, tag="rowa")
                            nc.sync.dma_start(rowa[:], t_rows_a[ch])
                            xls = sb.tile([128, 256], F32, tag="xls")
                            nc.gpsimd.indirect_dma_start(
                                out=xls[:], out_offset=None, in_=d_xl_all.opt(),
                                in_offset=bass.IndirectOffsetOnAxis(ap=srcc[:, :1], axis=0))
                            rep_ps = ps1.tile([32, 256], F32, space="PSUM", tag="reps")
                            nc.tensor.matmul(rep_ps[:], lhsT=ones1[:, :32], rhs=rowa[:],
                                             start=True, stop=True)
                            oh0 = sb.tile([32, 128], F32, tag="oh0")
                            nc.vector.tensor_scalar(out=oh0[:], in0=rep_ps[:, :128],
                                                    scalar1=iota32c[:, :1], scalar2=None,
                                                    op0=ALU.is_equal)
                            oh1 = sb.tile([32, 128], F32, tag="oh1")
                            nc.vector.tensor_scalar(out=oh1[:], in0=rep_ps[:, 128:],
                                                    scalar1=iota32c[:, :1], scalar2=None,
                                                    op0=ALU.is_equal)
                            st = sb.tile([128, 128], F32, tag="st")
                            nc.vector.tensor_scalar(out=st[:], in0=iota_f[:],
                                                    scalar1=trel[:, :1], scalar2=None,
                                                    op0=ALU.is_equal)
                            stm_ps = ps1.tile([128, 128], F32, space="PSUM", tag="stmps")
                            nc.tensor.transpose(stm_ps[:], st[:], ident[:])
                            stm = sb.tile([128, 128], F32, tag="stm")
                            nc.scalar.copy(stm[:], stm_ps[:])
                            v_ps = ps1.tile([128, 256], F32, space="PSUM", tag="vps")
                            nc.tensor.matmul(v_ps[:], lhsT=stm[:], rhs=xr_blk[:],
                                             start=True, stop=False)
                            nc.tensor.matmul(v_ps[:], lhsT=oh0[:], rhs=EA[:, lsl],
                                             start=False, stop=False)
                            nc.tensor.matmul(v_ps[:], lhsT=oh1[:], rhs=EB[:, lsl],
                                             start=False, stop=True)
                            v_sb = sb.tile([128, 256], F32, tag="vsb")
                            nc.vector.tensor_add(v_sb[:], v_ps[:], xls[:])
                            m_sb = sb.tile([128, 256], F32, tag="msb")
                            nc.vector.scalar_tensor_tensor(
                                out=m_sb[:], in0=v_sb[:], scalar=0.2, in1=v_sb[:],
                                op0=ALU.mult, op1=ALU.max)
                            am = sb.tile([128, 256], F32, tag="am")
                            nc.vector.tensor_tensor(out=am[:], in0=m_sb[:],
                                                    in1=att_sb[:, lsl], op=ALU.mult)
                            alpha = sb.tile([128, H], F32, tag="alpha")
                            nc.vector.reduce_sum(
                                alpha[:], am[:].rearrange("p (h c) -> p h c", h=H),
                                axis=AX)
                            p_t = sb.tile([128, H], F32, tag="pt")
                            nc.scalar.activation(p_t[:], alpha[:], ACTF.Exp)
                            y_t = sb.tile([128, 256], F32, tag="yt")
                            nc.vector.tensor_tensor(
                                out=y_t[:].rearrange("p (h c) -> p h c", h=H),
                                in0=xls[:].rearrange("p (h c) -> p h c", h=H),
                                in1=headb(p_t, H, C), op=ALU.mult)
                            nc.tensor.matmul(nd_ps[:], lhsT=st[:], rhs=y_t[:],
                                             start=(k == 0), stop=(k == KBLK - 1))
                            nc.tensor.matmul(den_ps[:], lhsT=st[:], rhs=p_t[:],
                                             start=(k == 0), stop=(k == KBLK - 1))
                            if debug and l == 0 and b == 0:
                                nc.sync.dma_start(dbg["dbg_v"][k], v_sb[:])
                                nc.sync.dma_start(dbg["dbg_alpha"][k, :, :H], alpha[:])
                                nc.sync.dma_start(dbg["dbg_xls"][k], xls[:])
                                nc.sync.dma_start(dbg["dbg_y"][k], y_t[:])
                                nc.sync.dma_start(dbg["dbg_p"][k, :, :H], p_t[:])
                        # self-loop + finalize
                        if debug and l == 0 and b == 0:
                            ndc = sb.tile([128, 256 + H], F32, tag="ndc")
                            nc.vector.tensor_copy(ndc[:, :256], nd_ps[:])
                            nc.vector.tensor_copy(ndc[:, 256:], den_ps[:])
                            nc.sync.dma_start(dbg["dbg_nd"][:, :256 + H], ndc[:])
                        xl_blk = sb.tile([128, 256], F32, tag="xlblk")
                        nc.sync.dma_start(xl_blk[:], d_xl[b * 128:(b + 1) * 128, :])
                        lft_t = sb.tile([128, 128], F32, tag="lftt")
                        nc.sync.dma_start(lft_t[:], d_lft[b * 128:(b + 1) * 128, :])
                        vl_ps = ps1.tile([128, 256], F32, space="PSUM", tag="vps")
                        nc.tensor.matmul(vl_ps[:], lhsT=lft_t[:], rhs=lew_sb[:, lsl],
                                         start=True, stop=True)
                        vl_sb = sb.tile([128, 256], F32, tag="vlsb")
                        nc.vector.tensor_add(vl_sb[:], vl_ps[:], xl_blk[:])
                        nc.vector.tensor_add(vl_sb[:], vl_sb[:], xr_blk[:])
                        ml = sb.tile([128, 256], F32, tag="msb")
                        nc.vector.scalar_tensor_tensor(
                            out=ml[:], in0=vl_sb[:], scalar=0.2, in1=vl_sb[:],
                            op0=ALU.mult, op1=ALU.max)
                        aml = sb.tile([128, 256], F32, tag="am")
                        nc.vector.tensor_tensor(out=aml[:], in0=ml[:],
                                                in1=att_sb[:, lsl], op=ALU.mult)
                        alpl = sb.tile([128, H], F32, tag="alpha")
                        nc.vector.reduce_sum(
                            alpl[:], aml[:].rearrange("p (h c) -> p h c", h=H), axis=AX)
                        pl = sb.tile([128, H], F32, tag="pt")
                        nc.scalar.activation(pl[:], alpl[:], ACTF.Exp)
                        yl = sb.tile([128, 256], F32, tag="yt")
                        nc.vector.tensor_tensor(
                            out=yl[:].rearrange("p (h c) -> p h c", h=H),
                            in0=xl_blk[:].rearrange("p (h c) -> p h c", h=H),
                            in1=headb(pl, H, C), op=ALU.mult)
                        numer_sb = sb.tile([128, 256], F32, tag="numsb")
                        nc.vector.tensor_add(numer_sb[:], nd_ps[:], yl[:])
                        den_sb = sb.tile([128, H], F32, tag="densb")
                        nc.vector.tensor_add(den_sb[:], den_ps[:], pl[:])
                        rden = sb.tile([128, H], F32, tag="rden")
                        nc.vector.reciprocal(rden[:], den_sb[:])
                        outb = sb.tile([128, 256], F32, tag="outb")
                        nc.vector.tensor_tensor(
                            out=outb[:].rearrange("p (h c) -> p h c", h=H),
                            in0=numer_sb[:].rearrange("p (h c) -> p h c", h=H),
                            in1=headb(rden, H, C), op=ALU.mult)
                        nc.vector.tensor_add(outb[:], outb[:], cvb_sb[:, lsl])
                        if b == NB - 1:
                            nc.vector.tensor_scalar(out=outb[:], in0=outb[:],
                                                    scalar1=mask97[:, :1], scalar2=None,
                                                    op0=ALU.mult)
                        nc.sync.dma_start(d_out[b * 128:(b + 1) * 128, :], outb[:])
                        sq = sb.tile([128, 256], F32, tag="sq")
                        nc.scalar.activation(sq[:], outb[:], ACTF.Square)
                        nc.tensor.matmul(sum_ps[:], lhsT=onesc[:], rhs=outb[:],
                                         start=(b == 0), stop=(b == NB - 1))
                        nc.tensor.matmul(sq_ps[:], lhsT=onesc[:], rhs=sq[:],
                                         start=(b == 0), stop=(b == NB - 1))

                    st_sb = sb.tile([1, 512], F32)
                    nc.vector.tensor_copy(st_sb[:, :256], sum_ps[:])
                    nc.vector.tensor_copy(st_sb[:, 256:], sq_ps[:])
                    nc.sync.dma_start(d_st_in[:], st_sb[:])
                    nc.gpsimd.collective_compute(
                        "AllReduce", ALU.add, ins=[d_st_in.opt()],
                        outs=[d_st_outs[l].opt()], replica_groups=[list(range(M))])

                # ---------------- pass B ----------------
                with (
                    tc.tile_pool(name=f"psB{l}", bufs=2, space="PSUM") as ps,
                    tc.tile_pool(name=f"sbB{l}", bufs=3) as sb,
                ):
                    stg = sb.tile([1, 512], F32)
                    nc.sync.dma_start(stg[:], d_st_outs[l].opt())
                    mu = sb.tile([1, 256], F32)
                    nc.vector.tensor_scalar(out=mu[:], in0=stg[:, :256], scalar1=1.0 / N,
                                            scalar2=None, op0=ALU.mult)
                    ex2 = sb.tile([1, 256], F32)
                    nc.vector.tensor_scalar(out=ex2[:], in0=stg[:, 256:], scalar1=1.0 / N,
                                            scalar2=None, op0=ALU.mult)
                    mu2 = sb.tile([1, 256], F32)
                    nc.vector.tensor_tensor(out=mu2[:], in0=mu[:], in1=mu[:], op=ALU.mult)
                    var = sb.tile([1, 256], F32)
                    nc.vector.tensor_tensor(out=var[:], in0=ex2[:], in1=mu2[:],
                                            op=ALU.subtract)
                    veps = sb.tile([1, 256], F32)
                    nc.vector.tensor_scalar(out=veps[:], in0=var[:], scalar1=1e-5,
                                            scalar2=None, op0=ALU.add)
                    sd = sb.tile([1, 256], F32)
                    nc.scalar.activation(sd[:], veps[:], ACTF.Sqrt)
                    rstd = sb.tile([1, 256], F32)
                    nc.vector.reciprocal(rstd[:], sd[:])
                    bng = sb.tile([1, 256], F32)
                    nc.sync.dma_start(bng[:], t_bng[l])
                    bnb = sb.tile([1, 256], F32)
                    nc.sync.dma_start(bnb[:], t_bnb[l])
                    rowAB = sb.tile([1, 512], F32)
                    nc.vector.tensor_tensor(out=rowAB[:, :256], in0=rstd[:], in1=bng[:],
                                            op=ALU.mult)
                    t3 = sb.tile([1, 256], F32)
                    nc.vector.tensor_tensor(out=t3[:], in0=mu[:], in1=rowAB[:, :256],
                                            op=ALU.mult)
                    nc.vector.tensor_tensor(out=rowAB[:, 256:], in0=bnb[:], in1=t3[:],
                                            op=ALU.subtract)
                    rab_ps = ps.tile([128, 512], F32, space="PSUM", tag="rabps")
                    nc.tensor.matmul(rab_ps[:], lhsT=ones1[:], rhs=rowAB[:],
                                     start=True, stop=True)
                    rab = sb.tile([128, 512], F32)
                    nc.scalar.copy(rab[:], rab_ps[:])

                    if l < nlayers - 1:
                        for b in range(NB):
                            op = sb.tile([128, 256], F32, tag="opre")
                            nc.sync.dma_start(op[:], d_out[b * 128:(b + 1) * 128, :])
                            xh = sb.tile([128, 256], F32, tag="xh")
                            nc.vector.tensor_tensor(out=xh[:], in0=op[:],
                                                    in1=rab[:, :256], op=ALU.mult)
                            nc.vector.tensor_add(xh[:], xh[:], rab[:, 256:])
                            xn2 = sb.tile([128, 256], F32, tag="xn2")
                            nc.scalar.activation(xn2[:], xh[:], ACTF.Lrelu)
                            emit_xlxr(xn2, l + 1, b, ps, sb)
                    else:
                        zer = sb.tile([128, 257], F32)
                        nc.any.memset(zer[:], 0.0)
                        for i in range(BPAD // 128):
                            nc.sync.dma_start(d_pool[i * 128:(i + 1) * 128, :], zer[:])
                        for b in range(NB):
                            op = sb.tile([128, 256], F32, tag="opre")
                            nc.sync.dma_start(op[:], d_out[b * 128:(b + 1) * 128, :])
                            xh = sb.tile([128, 256], F32, tag="xh")
                            nc.vector.tensor_tensor(out=xh[:], in0=op[:],
                                                    in1=rab[:, :256], op=ALU.mult)
                            nc.vector.tensor_add(xh[:], xh[:], rab[:, 256:])
                            brl = sb.tile([128, 1], F32, tag="brl")
                            nc.sync.dma_start(brl[:], t_brel[b])
                            pix = sb.tile([128, 1], I32, tag="pix")
                            nc.sync.dma_start(pix[:], t_pidx[b])
                            ohp = sb.tile([128, 128], F32, tag="ohp")
                            nc.vector.tensor_scalar(out=ohp[:], in0=iota_f[:],
                                                    scalar1=brl[:, :1], scalar2=None,
                                                    op0=ALU.is_equal)
                            pool_ps = ps.tile([128, 257], F32, space="PSUM", tag="poolps")
                            nc.tensor.matmul(pool_ps[:, :256], lhsT=ohp[:], rhs=xh[:],
                                             start=True, stop=True)
                            nc.tensor.matmul(pool_ps[:, 256:], lhsT=ohp[:], rhs=onesc[:],
                                             start=True, stop=True)
                            pool_sb = sb.tile([128, 257], F32, tag="poolsb")
                            nc.scalar.copy(pool_sb[:], pool_ps[:])
                            nc.gpsimd.indirect_dma_start(
                                out=d_pool.opt(), in_=pool_sb[:], in_offset=None,
                                out_offset=bass.IndirectOffsetOnAxis(ap=pix[:, :1], axis=0),
                                compute_op=ALU.add)

            # ---------------- pooling RS + MLP ----------------
            if nlayers == L:
                if debug:
                    nc.sync.dma_start(dbg["dbg_out"][:], d_out.opt())
                    nc.sync.dma_start(dbg["dbg_pool"][:], d_pool.opt())
                nc.gpsimd.collective_compute(
                    "ReduceScatter", ALU.add, ins=[d_pool[:B, :]],
                    outs=[d_pool_rs.opt()], replica_groups=[list(range(M))])
                with (
                    tc.tile_pool(name="psM", bufs=2, space="PSUM") as ps,
                    tc.tile_pool(name="sbM", bufs=3) as sb,
                    tc.tile_pool(name="wM", bufs=1) as wp,
                ):
                    w1s = [wp.tile([128, 1024], F32, name=f"w1_{i}") for i in range(2)]
                    for i in range(2):
                        nc.sync.dma_start(w1s[i][:], t_w1[128 * i:128 * (i + 1), :])
                    w2s = [wp.tile([128, 1024], F32, name=f"w2_{i}") for i in range(8)]
                    for i in range(8):
                        nc.sync.dma_start(w2s[i][:], t_w2[128 * i:128 * (i + 1), :])
                    w3s = [wp.tile([128, 512], F32, name=f"w3_{i}") for i in range(8)]
                    for i in range(8):
                        nc.sync.dma_start(w3s[i][:], t_w3[128 * i:128 * (i + 1), :])
                    w4s = [wp.tile([128, NCLS], F32, name=f"w4_{i}") for i in range(4)]
                    for i in range(4):
                        nc.sync.dma_start(w4s[i][:], t_w4[128 * i:128 * (i + 1), :])
                    b1s = wp.tile([128, 1024], F32)
                    nc.sync.dma_start(b1s[:], t_b1[:])
                    b2s = wp.tile([128, 1024], F32)
                    nc.sync.dma_start(b2s[:], t_b2[:])
                    b3s = wp.tile([128, 512], F32)
                    nc.sync.dma_start(b3s[:], t_b3[:])
                    b4s = wp.tile([128, NCLS], F32)
                    nc.sync.dma_start(b4s[:], t_b4[:])

                    def transpose_all(src, width, ps, sb, tag):
                        outs = []
                        for i in range(width // 128):
                            tp = ps.tile([128, 128], F32, space="PSUM", tag="tpM")
                            nc.tensor.transpose(tp[:], src[:, 128 * i:128 * (i + 1)],
                                                ident[:])
                            tsb = sb.tile([128, 128], F32, tag="ts" + tag)
                            nc.scalar.copy(tsb[:], tp[:])
                            outs.append(tsb)
                        return outs

                    for i in range(GPC // 128):
                        pc = sb.tile([128, 257], F32, tag="pc")
                        nc.sync.dma_start(pc[:], d_pool_rs[128 * i:128 * (i + 1), :])
                        cm = sb.tile([128, 1], F32, tag="cm")
                        nc.vector.tensor_scalar(out=cm[:], in0=pc[:, 256:], scalar1=1.0,
                                                scalar2=None, op0=ALU.max)
                        rc = sb.tile([128, 1], F32, tag="rc")
                        nc.vector.reciprocal(rc[:], cm[:])
                        g = sb.tile([128, 256], F32, tag="g")
                        nc.vector.tensor_scalar(out=g[:], in0=pc[:, :256],
                                                scalar1=rc[:, :1], scalar2=None,
                                                op0=ALU.mult)
                        gT = transpose_all(g, 256, ps, sb, "g")
                        h1 = sb.tile([128, 1024], F32, tag="h1")
                        for nh in range(2):
                            hp = ps.tile([128, 512], F32, space="PSUM", tag="hp")
                            for kk in range(2):
                                nc.tensor.matmul(
                                    hp[:], lhsT=gT[kk][:],
                                    rhs=w1s[kk][:, 512 * nh:512 * (nh + 1)],
                                    start=(kk == 0), stop=(kk == 1))
                            nc.vector.tensor_add(h1[:, 512 * nh:512 * (nh + 1)], hp[:],
                                                 b1s[:, 512 * nh:512 * (nh + 1)])
                        nc.scalar.activation(h1[:], h1[:], ACTF.Relu)
                        h1T = transpose_all(h1, 1024, ps, sb, "h1")
                        h2 = sb.tile([128, 1024], F32, tag="h2")
                        for nh in range(2):
                            hp = ps.tile([128, 512], F32, space="PSUM", tag="hp")
                            for kk in range(8):
                                nc.tensor.matmul(
                                    hp[:], lhsT=h1T[kk][:],
                                    rhs=w2s[kk][:, 512 * nh:512 * (nh + 1)],
                                    start=(kk == 0), stop=(kk == 7))
                            nc.vector.tensor_add(h2[:, 512 * nh:512 * (nh + 1)], hp[:],
                                                 b2s[:, 512 * nh:512 * (nh + 1)])
                        nc.scalar.activation(h2[:], h2[:], ACTF.Relu)
                        h2T = transpose_all(h2, 1024, ps, sb, "h2")
                        h3 = sb.tile([128, 512], F32, tag="h3")
                        hp = ps.tile([128, 512], F32, space="PSUM", tag="hp")
                        for kk in range(8):
                            nc.tensor.matmul(hp[:], lhsT=h2T[kk][:], rhs=w3s[kk][:],
                                             start=(kk == 0), stop=(kk == 7))
                        nc.vector.tensor_add(h3[:], hp[:], b3s[:])
                        nc.scalar.activation(h3[:], h3[:], ACTF.Relu)
                        h3T = transpose_all(h3, 512, ps, sb, "h3")
                        yp = ps.tile([128, NCLS], F32, space="PSUM", tag="hp")
                        for kk in range(4):
                            nc.tensor.matmul(yp[:], lhsT=h3T[kk][:], rhs=w4s[kk][:],
                                             start=(kk == 0), stop=(kk == 3))
                        yo = sb.tile([128, NCLS], F32, tag="yo")
                        nc.vector.tensor_add(yo[:], yp[:], b4s[:])
                        nc.sync.dma_start(out_y[128 * i:128 * (i + 1), :], yo[:])
            else:
                if debug:
                    nc.sync.dma_start(dbg["dbg_out"][:], d_out.opt())
                zo = cst.tile([128, NCLS], F32)
                nc.any.memset(zo[:], 0.0)
                for i in range(GPC // 128):
                    nc.sync.dma_start(out_y[128 * i:128 * (i + 1), :], zo[:])

    nc.compile()
    return nc


# ---------------------------------------------------------------- entry point
def kernel(**inputs) -> np.ndarray:
    in_maps, KBLK = host_prep(inputs)
    key = ("full", KBLK)
    if key not in _cache:
        _cache[key] = build(KBLK)
    nc = _cache[key]
    res = run_bass_kernel_spmd(nc, in_maps, list(range(M)))
    return np.concatenate([res.results[c]["out_y"] for c in range(M)], axis=0)


def run_partial(inputs, nlayers, debug=True):
    in_maps, KBLK = host_prep(inputs)
    key = (nlayers, KBLK, debug)
    if key not in _cache:
        _cache[key] = build(KBLK, nlayers=nlayers, debug=debug)
    nc = _cache[key]
    return run_bass_kernel_spmd(nc, in_maps, list(range(M)))



# revision 2
# speedup vs baseline: 7.1613x; 7.1613x over previous
"""TRN2 Bass kernel for nn_GAT_73950746902569 — instruction-count-minimized v2.

Backend charges ~40-90us per instruction nearly independent of data size, so
the design maximizes work per instruction: multi-row indirect gathers, wide
strided vector ops over superblocks (7 blocks x 128 targets), xbar bf16
transposes, per-pair edge-feature tables, and balanced edge packing.
"""
import numpy as np
import ml_dtypes

import concourse.bass as bass
import concourse.bacc as bacc
import concourse.mybir as mybir
import concourse.tile as tile
from concourse.bass_utils import run_bass_kernel_spmd

N, E, B = 100000, 200000, 4096
HID, EDIM, HEADS, L, NCLS = 256, 64, 8, 4, 3
M = 8
NPC = N // M            # 12500
NB = 98
NPAD = NB * 128         # 12544
SBW = 7                 # blocks per superblock
NSB = NB // SBW         # 14
GPC = B // M            # 512
BPAD = 4224
NPAIR = 484             # 22*22
P = 128

F32 = mybir.dt.float32
BF16 = mybir.dt.bfloat16
I32 = mybir.dt.int32
ALU = mybir.AluOpType
ACTF = mybir.ActivationFunctionType
AX = mybir.AxisListType.X

_cache = {}


def _bits(a):
    """[n] uint -> [n,8] f32 bits MSB-first."""
    return (((np.asarray(a)[:, None] >> np.arange(7, -1, -1)) & 1)
            .astype(np.float32))


def _bits_rows(a):
    """[n,k] -> [n,8k] f32 MSB-first per byte."""
    a = np.asarray(a)
    bits = ((a[:, :, None] >> np.arange(7, -1, -1)) & 1)
    return bits.reshape(a.shape[0], -1).astype(np.float32)


def _rep(v, n=128):
    v = np.asarray(v, np.float32)
    return np.broadcast_to(v[None, :], (n, v.shape[-1])).copy()


def _pack_core(deg):
    """Assign NPC local nodes to (block, lane), balancing per-block edge
    counts within each superblock toward multiples of 128.

    Returns pos[NPC] (padded position 0..NPAD-1) and per-(sb, j) edge counts.
    Within sb 13 the pad lanes land at the end of block j=6.
    """
    pos = np.empty(NPC, np.int64)
    keb = np.zeros((NSB, SBW), np.int64)
    for g in range(NSB):
        lo, hi = g * 896, min((g + 1) * 896, NPC)
        nodes = np.arange(lo, hi)
        d = deg[lo:hi]
        order = np.argsort(-d, kind="stable")
        nodes, d = nodes[order], d[order]
        nreal = hi - lo
        tot = int(d.sum())
        q = max((tot + 127) // 128, SBW)
        base, extra = q // SBW, q % SBW
        slots = [base + (1 if j < extra else 0) for j in range(SBW)]
        # per-block node capacity: 128, except short last sb spreads evenly
        caps = [128] * SBW
        if nreal < 896:
            caps[SBW - 1] = nreal - 128 * (SBW - 1)
        used = np.zeros(len(nodes), bool)
        blk_nodes = []
        for j in range(SBW):
            capn = caps[j]
            cape = 128 * slots[j]
            take = []
            s = 0
            # big nodes first while they fit in edge budget
            for i in range(len(nodes)):
                if used[i] or len(take) == capn:
                    continue
                if s + d[i] <= cape or j == SBW - 1:
                    used[i] = True
                    take.append(i)
                    s += d[i]
            if len(take) < capn:
                for i in range(len(nodes) - 1, -1, -1):
                    if used[i]:
                        continue
                    used[i] = True
                    take.append(i)
                    s += d[i]
                    if len(take) == capn:
                        break
            blk_nodes.append((np.array(take, np.int64), s))
        # order blocks by edge count desc for cross-core slot alignment
        order_j = sorted(range(SBW), key=lambda j: -blk_nodes[j][1])
        if nreal < 896:  # keep the short block last
            order_j = [j for j in order_j if caps[j] == 128] + \
                      [j for j in order_j if caps[j] != 128]
        for newj, oldj in enumerate(order_j):
            take, s = blk_nodes[oldj]
            b = g * SBW + newj
            pos[nodes[take]] = b * 128 + np.arange(len(take))
            keb[g, newj] = s
    return pos, keb


def host_prep(inputs):
    x = np.asarray(inputs["x"])
    edge_index = np.asarray(inputs["edge_index"])
    edge_attr = np.asarray(inputs["edge_attr"])
    batch = np.asarray(inputs["batch"])

    src, tgt = edge_index[0].astype(np.int64), edge_index[1].astype(np.int64)
    pair = (edge_attr[:, 0] * 22 + edge_attr[:, 1]).astype(np.int64)

    # ---- weight-derived tables (shared across cores) ----
    atom_emb = np.asarray(inputs["atom_emb"], np.float32)        # [120,128]
    alw = np.asarray(inputs["atom_lin_w"], np.float32)           # [56,128]
    alb = np.asarray(inputs["atom_lin_b"], np.float32)           # [128]
    edge_emb = np.asarray(inputs["edge_emb"], np.float32)        # [22,64]
    elw = np.asarray(inputs["edge_lin_w"], np.float32)           # [8,64]
    elb = np.asarray(inputs["edge_lin_b"], np.float32)           # [64]
    lin_l_w = np.asarray(inputs["lin_l_w"], np.float32)
    lin_r_w = np.asarray(inputs["lin_r_w"], np.float32)
    lin_e_w = np.asarray(inputs["lin_e_w"], np.float32)

    a0g, a1g = np.meshgrid(np.arange(22), np.arange(22), indexing="ij")
    ef_pairs = np.concatenate(
        [edge_emb[a0g.ravel()], _bits(a1g.ravel()) @ elw + elb],
        axis=1).astype(np.float32)                               # [484,128]
    eft = np.zeros((NPAIR, 132), np.float32)
    eft[:, :128] = ef_pairs
    eft[:, 128] = 1.0
    eetab_pairs = np.stack(
        [ef_pairs @ lin_e_w[l] for l in range(L)]).astype(np.float32)

    W = {}
    W["eft"] = eft
    W["eetab_pairs"] = eetab_pairs                              # [L,484,256]
    W["wcat"] = np.stack([
        np.stack([np.concatenate([lin_l_w[l, 128 * h:128 * (h + 1)],
                                  lin_r_w[l, 128 * h:128 * (h + 1)]], axis=1)
                  for h in range(2)]) for l in range(L)
    ]).astype(ml_dtypes.bfloat16)                               # [L,2,128,512]
    W["xlr_b"] = np.stack([
        _rep(np.concatenate([np.asarray(inputs["lin_l_b"])[l],
                             np.asarray(inputs["lin_r_b"])[l]]))
        for l in range(L)])                                     # [L,128,512]
    W["lew"] = lin_e_w.astype(ml_dtypes.bfloat16)               # [L,128,256]
    W["att_rep"] = np.stack([_rep(np.asarray(inputs["att"])[l])
                             for l in range(L)])
    W["convb_rep"] = np.stack([_rep(np.asarray(inputs["conv_b"])[l])
                               for l in range(L)])
    W["bng"] = np.asarray(inputs["bn_g"], np.float32)[:, None, :]
    W["bnb"] = np.asarray(inputs["bn_b"], np.float32)[:, None, :]
    aemb_pad = np.zeros((128, 128), np.float32)
    aemb_pad[:120] = atom_emb
    W["aemb_pad"] = aemb_pad
    W["alw"] = alw
    W["alb_col"] = alb[:, None].astype(np.float32)              # [128,1]
    W["iota"] = np.broadcast_to(np.arange(128, dtype=np.float32)[None, :],
                                (128, 128)).copy()
    mask97 = np.zeros((128, 1), np.float32)
    W["mask97"] = mask97  # filled per-core? same for all: lanes < 84
    mask97[:NPC - 97 * 128] = 1.0
    for k in ("w1", "w2", "w3", "w4"):
        W[k] = np.asarray(inputs[k], np.float32).astype(ml_dtypes.bfloat16)
    for k, wd in (("b1", 1024), ("b2", 1024), ("b3", 512), ("b4", NCLS)):
        W[k + "_rep"] = _rep(np.asarray(inputs[k]))

    # ---- per-core packing ----
    deg_all = np.bincount(tgt, minlength=N)
    pos_all = np.empty(N, np.int64)
    kebs = []
    for c in range(M):
        sl = slice(c * NPC, (c + 1) * NPC)
        pos, keb = _pack_core(deg_all[sl])
        pos_all[sl] = pos
        kebs.append(keb)
    Ktab = np.maximum.reduce([(k + 127) // 128 for k in kebs])   # [NSB,SBW]
    gpad = (np.arange(N) // NPC) * NPAD + pos_all                # global padded row

    SE = int(Ktab.sum())
    S = SE + NB
    # global slot col layout: per sb: edge slots (block j asc, k asc), then
    # 7 self slots. Edge-slot-only index for trel/st.
    sb_e0 = np.zeros(NSB + 1, np.int64)    # edge-slot base per sb
    for g in range(NSB):
        sb_e0[g + 1] = sb_e0[g] + Ktab[g].sum()

    src_idx = np.zeros((M, 128, S), np.int32)
    tgt_idx = np.zeros((M, 128, S), np.int32)
    ee_idx = np.zeros((M, 128, S), np.int32)
    trel = np.full((M, 128, SE), 200.0, np.float32)
    x0row = np.zeros((M, 1, NPAD), np.float32)
    bitsT = np.zeros((M, 56, NPAD), np.float32)
    brel = np.full((M, 128, NB), 200.0, np.float32)
    pidx = np.zeros((M, 128, NB), np.int32)

    for c in range(M):
        sl = slice(c * NPC, (c + 1) * NPC)
        pos = pos_all[sl]
        # node-indexed uploads in padded layout
        x0row[c, 0, pos] = x[sl][:, 0].astype(np.float32)
        bitsT[c][:, pos] = _bits_rows(x[sl][:, 1:8]).T
        bc = batch[sl]
        for b in range(NB):
            lanes = np.where(pos // 128 == b)[0]
            lane_of = pos[lanes] % 128
            gb = int(bc[lanes].min()) if len(lanes) else 0
            assert len(lanes) == 0 or int(bc[lanes].max()) - gb < 128
            brel[c, lane_of, b] = bc[lanes] - gb
            pidx[c, :, b] = gb + np.arange(128)
        # edges of this core grouped by target block
        em = (tgt >= c * NPC) & (tgt < (c + 1) * NPC)
        et, es, ep = tgt[em] - c * NPC, src[em], pair[em]
        epos = pos[et]
        eb = epos // 128
        order = np.argsort(eb, kind="stable")
        et, es, ep, epos, eb = et[order], es[order], ep[order], epos[order], eb[order]
        starts = np.searchsorted(eb, np.arange(NB + 1))
        for g in range(NSB):
            col = sb_e0[g]
            for j in range(SBW):
                b = g * SBW + j
                e0, e1 = starts[b], starts[b + 1]
                cnt = e1 - e0
                K = int(Ktab[g, j])
                assert cnt <= K * 128, (c, g, j, cnt, K)
                for k in range(K):
                    lo = e0 + k * 128
                    hi = min(e1, lo + 128)
                    mlen = max(hi - lo, 0)
                    if mlen > 0:
                        src_idx[c, :mlen, col] = gpad[es[lo:hi]]
                        tgt_idx[c, :mlen, col] = epos[lo:hi]
                        ee_idx[c, :mlen, col] = ep[lo:hi]
                        trel[c, :mlen, col] = (epos[lo:hi] % 128).astype(np.float32)
                    col += 1
    colmap_edge = np.zeros(SE, np.int64)
    colmap_self = np.zeros(NB, np.int64)
    cc = 0
    for g in range(NSB):
        ne = int(Ktab[g].sum())
        for i in range(ne):
            colmap_edge[sb_e0[g] + i] = cc + i
        for j in range(SBW):
            colmap_self[g * SBW + j] = cc + ne + j
        cc += ne + SBW
    assert cc == S

    src_idx2 = np.zeros((M, 128, S), np.int32)
    tgt_idx2 = np.zeros((M, 128, S), np.int32)
    ee_idx2 = np.zeros((M, 128, S), np.int32)
    src_idx2[:, :, colmap_edge] = src_idx[:, :, :SE]
    tgt_idx2[:, :, colmap_edge] = tgt_idx[:, :, :SE]
    ee_idx2[:, :, colmap_edge] = ee_idx[:, :, :SE]
    lane = np.arange(128, dtype=np.int32)
    for c in range(M):
        for b in range(NB):
            rows = b * 128 + lane
            src_idx2[c, :, colmap_self[b]] = c * NPAD + rows
            tgt_idx2[c, :, colmap_self[b]] = rows
            ee_idx2[c, :, colmap_self[b]] = NPAIR + rows

    in_maps = []
    for c in range(M):
        im = dict(W)
        im["src_idx"] = src_idx2[c]
        im["tgt_idx"] = tgt_idx2[c]
        im["ee_idx"] = ee_idx2[c]
        im["trel"] = trel[c]
        im["x0row"] = x0row[c]
        im["bitsT"] = bitsT[c]
        im["brel"] = brel[c]
        im["pidx"] = pidx[c]
        in_maps.append(im)

    spec = {"Ktab": Ktab.tolist(), "SE": SE, "S": S}
    return in_maps, spec, pos_all
